# revision 57
# baseline (speedup 1.0000x reference)
"""Single-head attention (B=4, S=2048, E=1024, fp32) on 8 trn2 NeuronCores.

Sharding: (batch, key-half) -> 8 shards. Core c handles batch c//2 and half
h = c%2: keys/values AND queries [h*1024, (h+1)*1024) of x[b]. Per core:
  - Q/K/V projections for its own 1024 rows only (Wq/Wk applied without the
    1/sqrt(E) scale; outputs quantized straight to fp8e4m3, which needs the
    natural value range -- the scale is applied later inside the exp).
  - Q^T halves are exchanged within each core pair via a DRAM AllGather
    (replica_groups [[0,1],[2,3],[4,5],[6,7]]), landing in global query
    order; the exchange overlaps the K and V projections.
  - scores^T = K^T-stationary x Q^T-moving in fp8 DoubleRow perf mode
    (2 f-tiles contracted per matmul at 0.5 cycles/row), exp via the scalar
    engine (scale=1/32) into a bf16 pt tile kept resident in SBUF.
  - V carries an extra ones column (padded tile), so the O matmul
    O_h = pt^T @ [V_h | 1] yields the softmax denominators rs_h as column
    1024 -- no separate rowsum pass on the PE.
  - Outputs: unnormalized O_h (bf16) + rs_h; the host combines
    out = (O_0 + O_1) / (rs_0 + rs_1) + bv  (the V bias commutes with the
    softmax average, so it is added once on the host).

Matmul dtype/perf notes (measured on HW):
  - bf16 matmuls hit the 215ns/512-col streaming floor ONLY when the
    stationary operand is reused across consecutive matmuls; swapping the
    stationary every matmul exposes ~45ns of FWL LDWEIGHTS. All loops keep
    each stationary for 2-4 consecutive matmuls.
  - SBUF tiles must keep 16-byte-aligned free-dim rows; the V tile is
    padded to 1040 columns for this.
  - No warmup burst: the first real (cold) matmuls do the HAM clock ramp
    themselves (2.4GHz only after ~3.4us of continuous PE activity; cold
    = 1.2GHz). Cold pass-0 consumption self-paces with DMA delivery, and
    having no burst removes the warmup-vs-data race that re-throttled
    the clock on late-DMA runs. Long all-partition bursts also risk the
    sustained-power P0 downclock (2.0GHz whole-kernel).
  - Engine DMA queues: sync + scalar have ~150GB/s hardware-DGE rings;
    gpsimd is a ~34-85GB/s software queue. The DGE rings come up at
    ~8.9us (framework preamble); nothing big lands before ~10.5us.
  - The tensor engine pauses ~430ns every ~10.8us (firmware tick);
    unavoidable.

Input DMA schedule (arrival must match consumption). Each hwdge trigger
costs ~0.6us of engine time per 128 per-partition descriptors and the
ring admits only a few outstanding requests, so the schedule uses FEW,
GROUPED DMAs (weights are laid out partition-major per 4-f-tile group
host-side so one DMA covers an (f-group x e-range) block):
  - sync: wq f0-3 in four groups of increasing size in pass-0
    consumption order, wq f4-7 in two, then wk (2) -- 8 weight triggers
    total instead of ~50.
  - scalar: xt e-tiles 0-6 (e0 split in half for the earliest start),
    then wv (idle ring, needed only at ~69us; keeping it off sync pulls
    wk ~7us earlier for K-proj margin).
  - gpsimd: xt e-tile 7, bq/bk/ones, and the AllGather collectives.
  - Q projection runs as 2 passes of 4 f-tiles with the e-loop OUTER
    (8 PSUM banks), so pass 0 consumes xt e-tiles at ~1.7us/tile -- the
    ring delivery rate -- instead of needing all of xt in 3.4us.
  - The pair AllGather is SPLIT: each 4-f-tile half ships right after
    its Q-proj pass (stage via scalar, readback via sync hwdge rings).
    The collective runs at only ~63-86GB/s plus a partner barrier; a
    single post-Q AG had ~zero margin before the scores phase and
    stalled the PE up to 17us on jittery runs.
  - Output ou writes rotate across both hwdge rings per 256/512-col
    chunk so the 4MB drains inline with the O-phase compute.

SBUF layout (per core):
  xt [128, 8e, 1024] bf16  own 1024 rows of x[b]^T (moving + V stationary)
  w  [128, 8e, 128]  bf16  Wq^T / Wk^T stationary tiles (streamed)
  qt [128, 8f, 1024] fp8   own Q^T half; qt_g [128, 8f, 2048] gathered
  kt [128, 8f, 1024] fp8   K^T (scores stationary)
  wv [128, 8e, 1024] bf16  Wv^T (V-proj moving)
  v  [128, 8k, 1040] bf16  [V | 1 | pad] (O moving)
  pt [128, 8k, 2048] bf16  exp(scores^T) (O stationary)
"""

import numpy as np

P = 128


def _emit(nc, E=1024, S=2048, SK=1024):
    import concourse.mybir as mybir
    import concourse.tile as tile

    f32 = mybir.dt.float32
    f32r = mybir.dt.float32r
    bf16 = mybir.dt.bfloat16
    fp8 = mybir.dt.float8e4
    ACT = mybir.ActivationFunctionType

    ET = E // P     # e/f tiles (8)
    QT = S // P     # q tiles (16)
    KT = SK // P    # k tiles (8)
    NQC = S // 512  # q chunks (4)
    NKC = SK // 512  # k chunks (2)
    NFC = E // 512  # f chunks (2)

    # wq/wk are laid out partition-major per 4-f-tile group so a SINGLE DMA
    # trigger can deliver a whole (f-group x e-range) block: each hwdge
    # trigger costs ~0.6us of engine time (descriptor generation) and the
    # ring admits only a few outstanding requests, so many small triggers
    # serialize at ~0.65us each and delay the first matmul by ~3us
    xt8 = nc.dram_tensor("xt8", [ET, P, SK], bf16, kind="ExternalInput")
    wq8 = nc.dram_tensor("wq8", [2, P, 4, ET, P], bf16, kind="ExternalInput")
    wk8 = nc.dram_tensor("wk8", [2, P, 4, ET, P], bf16, kind="ExternalInput")
    wv8 = nc.dram_tensor("wv8", [P, ET, E], bf16, kind="ExternalInput")
    bq8 = nc.dram_tensor("bq8", [P, ET], f32, kind="ExternalInput")
    bk8 = nc.dram_tensor("bk8", [P, ET], f32, kind="ExternalInput")
    ones8 = nc.dram_tensor("ones8", [P, 8], bf16, kind="ExternalInput")
    ou = nc.dram_tensor("ou", [S, E], bf16, kind="ExternalOutput")
    rs = nc.dram_tensor("rs", [P, QT], f32, kind="ExternalOutput")

    groups = [[2 * i, 2 * i + 1] for i in range(4)]

    with tile.TileContext(nc) as tc:
        with (
            tc.tile_pool(name="dramp", bufs=1, space="DRAM") as dramp,
            tc.tile_pool(name="psum", bufs=8, space="PSUM") as psum,
            tc.tile_pool(name="small", bufs=1) as small,
            tc.tile_pool(name="persist", bufs=1) as pers,
            tc.tile_pool(name="obuf", bufs=3) as obp,
        ):
            # small tiles ride gpsimd: their triggers would otherwise burn
            # ~1.3us of sync trigger-head time in front of the critical wq
            ones_t = small.tile([P, 8], bf16, tag="ones")
            nc.gpsimd.dma_start(ones_t[:], ones8[:])
            bq_t = small.tile([P, ET], f32, tag="bq")
            nc.gpsimd.dma_start(bq_t[:], bq8[:])
            bk_t = small.tile([P, ET], f32, tag="bk")
            nc.gpsimd.dma_start(bk_t[:], bk8[:])
            rs_sb = small.tile([P, QT], f32, tag="rssb")

            qt_g = pers.tile([P, ET, S], fp8, tag="qtg")
            kt_t = pers.tile([P, ET, SK], fp8, tag="kt")
            v_t = pers.tile([P, KT, E + 16], bf16, tag="v")

            with (
                tc.tile_pool(name="ph1", bufs=1) as ph1,
            ):
                xt_t = ph1.tile([P, ET, SK], bf16, tag="xt")
                wv_t = ph1.tile([P, ET, E], bf16, tag="wv")
                qt_t = ph1.tile([P, ET, SK], fp8, tag="qt")
                wq_g = [
                    ph1.tile([P, 4, ET, P], bf16, tag=f"wq{g}", name=f"wq_g{g}")
                    for g in range(2)
                ]
                wk_g = [
                    ph1.tile([P, 4, ET, P], bf16, tag=f"wk{g}", name=f"wk_g{g}")
                    for g in range(2)
                ]
                # per-half AG staging: the pair AllGather runs at only
                # ~63-86GB/s plus a partner-sync barrier, so one 1MB AG
                # issued after the full Q projection has ~zero margin before
                # the scores phase needs qt_g (17us PE stall on unlucky
                # runs). Shipping each 4-f-tile half as soon as its Q-proj
                # pass finishes starts the exchange ~16us earlier.
                qh_h = [
                    dramp.tile(
                        [P, ET // 2, SK], fp8, tag=f"qh{h}", name=f"qh_d{h}"
                    )
                    for h in range(2)
                ]
                qg_h = [
                    dramp.tile(
                        [2, P, ET // 2, SK], fp8, tag=f"qg{h}", name=f"qg_d{h}"
                    )
                    for h in range(2)
                ]

                # (no warmup burst: the first real matmuls do the HAM clock
                # ramp themselves -- cold pass-0 consumption, 8 MMs/e-tile at
                # 430ns, self-paces with the ~1.7us/e-tile DMA delivery, and
                # dropping the burst removes the warmup-vs-data race that
                # re-throttled the clock on late-DMA runs)

                # ---- input DMA ----
                # sync: wq f0-3 in four grouped (all-f x e-pair) DMAs in
                #   pass-0 consumption order, then wq f4-7 / wk / wv as one
                #   big DMA each -- 8 triggers total instead of ~50.
                # scalar: xt e-tiles 0-6 e-major (matches the Q-proj pass-0
                #   e-outer consumption rate of ~1.7us/e-tile); gpsimd takes
                #   xt e-tile 7 + the small tiles.
                # first two e-pair groups are small for the earliest start;
                # later blocks use e-quad groups (1KB descriptors, ~2x the
                # generation efficiency) so the serial ring stays ahead of
                # the pass-0/pass-1 consumption rate
                nc.sync.dma_start(wq_g[0][:, 0:2, 0:2, :], wq8[0][:, 0:2, 0:2, :])
                nc.sync.dma_start(wq_g[0][:, :, 2:4, :], wq8[0][:, :, 2:4, :])
                nc.sync.dma_start(wq_g[0][:, :, 4:8, :], wq8[0][:, :, 4:8, :])
                nc.sync.dma_start(wq_g[1][:, :, 0:4, :], wq8[1][:, :, 0:4, :])
                nc.sync.dma_start(wq_g[1][:, :, 4:8, :], wq8[1][:, :, 4:8, :])
                nc.sync.dma_start(wk_g[0][:], wk8[0])
                nc.sync.dma_start(wk_g[1][:], wk8[1])
                # xt: full e-tile transfers (2KB contiguous per partition); e0
                # is split so its first 512 columns land with the first wq
                nc.scalar.dma_start(xt_t[:, 0, 0:512], xt8[0, :, 0:512])
                nc.scalar.dma_start(xt_t[:, 0, 512:SK], xt8[0, :, 512:SK])
                # second wq opening group rides SCALAR between e0 and e1: the
                # two rings deliver the cold-window operands in parallel,
                # closing the last ~2.9us of ramp gaps
                nc.scalar.dma_start(
                    wq_g[0][:, 2:4, 0:2, :], wq8[0][:, 2:4, 0:2, :]
                )
                for e in range(1, ET):
                    ring = nc.gpsimd if e == ET - 1 else nc.scalar
                    ring.dma_start(xt_t[:, e], xt8[e])
                # wv rides scalar after xt (idle ring, needed only at ~69us);
                # keeping it off sync pulls wk ~7us earlier for K-proj margin
                nc.scalar.dma_start(wv_t[:], wv8[:])

                # ---- Q projection, own query half only (the first SK
                # permuted columns = global queries [h*SK,(h+1)*SK)).
                # 2 passes of 4 f-tiles with the e-loop OUTER: pass 0 consumes
                # xt e-tiles at ~1.7us/tile, the rate the DMA ring delivers
                # them, so the PE starts ~6us earlier without starving ----
                for half in range(2):
                    fb = 4 * half
                    psq = [
                        psum.tile([P, 512], f32, tag="mm", name=f"q{fb}_{fi}{qc}")
                        for fi in range(4)
                        for qc in range(NKC)
                    ]
                    for e in range(ET):
                        for fi in range(4):
                            for qc in range(NKC):
                                nc.tensor.matmul(
                                    psq[fi * NKC + qc][:],
                                    wq_g[half][:, fi, e],
                                    xt_t[:, e, qc * 512 : (qc + 1) * 512],
                                    start=(e == 0),
                                    stop=(e == ET - 1),
                                )
                    for fi in range(4):
                        for qc in range(NKC):
                            nc.scalar.add(
                                qt_t[:, fb + fi, qc * 512 : (qc + 1) * 512],
                                psq[fi * NKC + qc][:],
                                bq_t[:, fb + fi : fb + fi + 1],
                            )
                    # stage this half to DRAM, pair-allgather into global
                    # query order, read back both cores' copies; overlaps
                    # with the rest of Q-proj and the K/V projections. Stage
                    # + readback ride the fast hwdge rings (scalar / sync,
                    # both idle here) -- gpsimd's ~34GB/s software queue was
                    # itself a several-us race
                    nc.scalar.dma_start(
                        qh_h[half][:], qt_t[:, fb : fb + 4]
                    )
                    nc.gpsimd.collective_compute(
                        "AllGather",
                        mybir.AluOpType.bypass,
                        replica_groups=groups,
                        ins=[qh_h[half][:]],
                        outs=[qg_h[half][:]],
                    )
                    for g in range(2):
                        nc.sync.dma_start(
                            qt_g[:, fb : fb + 4, g * SK : (g + 1) * SK],
                            qg_h[half][g],
                        )

                # ---- K projection (key half = first SK columns of xt) ----
                for f in range(ET):
                    ps2 = [
                        psum.tile([P, 512], f32, tag="mm", name=f"k{f}_{kc}")
                        for kc in range(NKC)
                    ]
                    for e in range(ET):
                        for kc in range(NKC):
                            nc.tensor.matmul(
                                ps2[kc][:],
                                wk_g[f // 4][:, f % 4, e],
                                xt_t[:, e, kc * 512 : (kc + 1) * 512],
                                start=(e == 0),
                                stop=(e == ET - 1),
                            )
                    for kc in range(NKC):
                        nc.scalar.add(
                            kt_t[:, f, kc * 512 : (kc + 1) * 512],
                            ps2[kc][:],
                            bk_t[:, f : f + 1],
                        )

                # ---- V projection: v[k, f] = sum_e xt[e, k] * wv[e, f] ----
                for kt in range(KT):
                    ps2 = [
                        psum.tile([P, 512], f32, tag="mm", name=f"v{kt}_{fc}")
                        for fc in range(NFC)
                    ]
                    for e in range(ET):
                        for fc in range(NFC):
                            nc.tensor.matmul(
                                ps2[fc][:],
                                xt_t[:, e, kt * P : (kt + 1) * P],
                                wv_t[:, e, fc * 512 : (fc + 1) * 512],
                                start=(e == 0),
                                stop=(e == ET - 1),
                            )
                    for fc in range(NFC):
                        nc.vector.tensor_copy(
                            v_t[:, kt, fc * 512 : (fc + 1) * 512], ps2[fc][:]
                        )
                    nc.vector.tensor_copy(v_t[:, kt, E : E + 1], ones_t[:, 0:1])

            with tc.tile_pool(name="ptp", bufs=1) as ptp:
                pt_t = ptp.tile([P, KT, S], bf16, tag="pt")

                # ---- scores^T (fp8 DoubleRow) + exp; rowsums in a second
                # pass so the PE never waits on the scalar exp ----
                # 4 q-chunks per stationary (vs 2): halves the DR LDWEIGHTS
                # exposure per 512-col matmul
                DR = mybir.MatmulPerfMode.DoubleRow
                NQC4 = S // 512
                scale = float(1.0 / np.sqrt(np.float32(E)))
                for kt in range(KT):
                    ps4 = [
                        psum.tile([P, 512], f32, tag="mm", name=f"s{kt}_{qc}")
                        for qc in range(NQC4)
                    ]
                    for fp in range(ET // 2):
                        for qc in range(NQC4):
                            nc.tensor.matmul(
                                ps4[qc][:],
                                kt_t[:, 2 * fp : 2 * fp + 2, kt * P : (kt + 1) * P],
                                qt_g[:, 2 * fp : 2 * fp + 2, qc * 512 : (qc + 1) * 512],
                                start=(fp == 0),
                                stop=(fp == ET // 2 - 1),
                                perf_mode=DR,
                            )
                    for qc in range(NQC4):
                        nc.scalar.activation(
                            pt_t[:, kt, qc * 512 : (qc + 1) * 512], ps4[qc][:],
                            ACT.Exp, scale=scale,
                        )

                # ---- O = pt^T @ v, unnormalized; store bf16 ----
                # output DMA rotates across both hwdge rings per chunk so the
                # 4MB of ou writes drain inline with compute; each chunk's
                # DMA fires as soon as its copy lands (short final drain)
                CH = [(0, 512), (512, 768), (768, E + 1)]
                out_rings = [nc.sync, nc.scalar]
                for qt in range(QT):
                    po = [
                        psum.tile([P, 512], f32, tag="mm", name=f"o{qt}_{fc}")
                        for fc in range(len(CH))
                    ]
                    for kt in range(KT):
                        for fc, (c0, c1) in enumerate(CH):
                            nc.tensor.matmul(
                                po[fc][:, : c1 - c0],
                                pt_t[:, kt, qt * P : (qt + 1) * P],
                                v_t[:, kt, c0:c1],
                                start=(kt == 0),
                                stop=(kt == KT - 1),
                            )
                    o_sb = obp.tile([P, E], bf16, tag="ob")
                    nc.vector.tensor_copy(o_sb[:, 0:512], po[0][:])
                    out_rings[qt % 2].dma_start(
                        ou[qt * P : (qt + 1) * P, 0:512], o_sb[:, 0:512]
                    )
                    nc.vector.tensor_copy(o_sb[:, 512:768], po[1][:, :256])
                    out_rings[(qt + 1) % 2].dma_start(
                        ou[qt * P : (qt + 1) * P, 512:768], o_sb[:, 512:768]
                    )
                    nc.vector.tensor_copy(o_sb[:, 768:E], po[2][:, :256])
                    nc.vector.tensor_copy(
                        rs_sb[:, qt : qt + 1], po[2][:, 256:257]
                    )
                    out_rings[qt % 2].dma_start(
                        ou[qt * P : (qt + 1) * P, 768:E], o_sb[:, 768:E]
                    )
                nc.sync.dma_start(rs[:], rs_sb[:])


_NC_CACHE = {}


def build_nc(E=1024, S=2048, SK=1024):
    key = (E, S, SK)
    if key in _NC_CACHE:
        return _NC_CACHE[key]
    import concourse.bacc as bacc

    nc = bacc.Bacc(None, target_bir_lowering=False)
    _emit(nc, E=E, S=S, SK=SK)
    nc.finalize()
    _NC_CACHE[key] = nc
    return nc


def _round_f32r(a):
    """Round fp32 to fp32r (tf32-like: 11 explicit mantissa bits, RNE)."""
    u = np.ascontiguousarray(a, np.float32).view(np.uint32)
    u = u + np.uint32(0x7FF) + ((u >> np.uint32(12)) & np.uint32(1))
    return (u & np.uint32(0xFFFFF000)).view(np.float32)


def make_in_maps(x, Wq, bq, Wk, bk, Wv, bv, E=1024, S=2048, SK=1024):
    """Host-side prep: per-core input dicts for run_bass_kernel_spmd."""
    import ml_dtypes

    bf16 = ml_dtypes.bfloat16
    ET = E // P
    scale = np.float32(1.0 / np.sqrt(np.float32(E)))
    x = np.asarray(x, np.float32)
    B = x.shape[0]
    n_half = S // SK

    def wtile(w):
        # [group, p(e), f_in_group, e_tile, c(f)] stationary blocks,
        # partition-major per 4-f-tile group so one DMA covers a whole
        # (f-group x e-range) block with matching iteration order
        return np.ascontiguousarray(
            np.asarray(w, np.float32)
            .reshape(2, 4, P, ET, P)
            .transpose(0, 4, 1, 3, 2)
        ).astype(bf16)

    wq8 = wtile(Wq)
    wk8 = wtile(Wk)
    # wv8[p, e, f] = Wv[f, e*128+p] (partition-major for a single DMA)
    wv8 = np.ascontiguousarray(
        np.asarray(Wv, np.float32).T.reshape(ET, P, E).transpose(1, 0, 2)
    ).astype(bf16)
    bq8 = np.ascontiguousarray(np.asarray(bq, np.float32).reshape(ET, P).T)
    bk8 = np.ascontiguousarray(np.asarray(bk, np.float32).reshape(ET, P).T)
    ones8 = np.ones((P, 8), bf16)

    in_maps = []
    for c in range(B * n_half):
        b, h = divmod(c, n_half)
        xt_half = x[b].T[:, h * SK : (h + 1) * SK]  # [E, SK]
        xt8 = np.ascontiguousarray(xt_half.reshape(ET, P, SK)).astype(bf16)
        in_maps.append(
            {
                "xt8": xt8,
                "wq8": wq8,
                "wk8": wk8,
                "wv8": wv8,
                "bq8": bq8,
                "bk8": bk8,
                "ones8": ones8,
            }
        )
    return in_maps


def kernel(x, Wq, bq, Wk, bk, Wv, bv):
    from concourse.bass_utils import run_bass_kernel_spmd

    E, S, SK = 1024, 2048, 1024
    x = np.asarray(x, np.float32)
    B = x.shape[0]
    n_half = S // SK
    nc = build_nc(E=E, S=S, SK=SK)
    in_maps = make_in_maps(x, Wq, bq, Wk, bk, Wv, bv, E=E, S=S, SK=SK)
    n_cores = len(in_maps)
    res = run_bass_kernel_spmd(nc, in_maps, list(range(n_cores)))

    bvf = np.asarray(bv, np.float32)
    out = np.empty((B, S, E), np.float32)
    for b in range(B):
        osum = None
        rsum = None
        for h in range(n_half):
            r = res.results[b * n_half + h]
            o_h = np.asarray(r["ou"]).astype(np.float32)
            rs_h = np.asarray(r["rs"]).astype(np.float32).T.reshape(S)
            osum = o_h if osum is None else osum + o_h
            rsum = rs_h if rsum is None else rsum + rs_h
        out[b] = osum / rsum[:, None] + bvf[None, :]
    return out



# revision 59
# speedup vs baseline: 1.0074x; 1.0074x over previous
"""Single-head attention (B=4, S=2048, E=1024, fp32) on 8 trn2 NeuronCores.

Sharding: (batch, key-half) -> 8 shards. Core c handles batch c//2 and half
h = c%2: keys/values AND queries [h*1024, (h+1)*1024) of x[b]. Per core:
  - Q/K/V projections for its own 1024 rows only (Wq/Wk applied without the
    1/sqrt(E) scale; outputs quantized straight to fp8e4m3, which needs the
    natural value range -- the scale is applied later inside the exp).
  - Q^T halves are exchanged within each core pair via a DRAM AllGather
    (replica_groups [[0,1],[2,3],[4,5],[6,7]]), landing in global query
    order; the exchange overlaps the K and V projections.
  - scores^T = K^T-stationary x Q^T-moving in fp8 DoubleRow perf mode
    (2 f-tiles contracted per matmul at 0.5 cycles/row), exp via the scalar
    engine (scale=1/32) into a bf16 pt tile kept resident in SBUF.
  - V carries an extra ones column (padded tile), so the O matmul
    O_h = pt^T @ [V_h | 1] yields the softmax denominators rs_h as column
    1024 -- no separate rowsum pass on the PE.
  - Outputs: unnormalized O_h (bf16) + rs_h; the host combines
    out = (O_0 + O_1) / (rs_0 + rs_1) + bv  (the V bias commutes with the
    softmax average, so it is added once on the host).

Matmul dtype/perf notes (measured on HW):
  - bf16 matmuls hit the 215ns/512-col streaming floor ONLY when the
    stationary operand is reused across consecutive matmuls; swapping the
    stationary every matmul exposes ~45ns of FWL LDWEIGHTS. All loops keep
    each stationary for 2-4 consecutive matmuls.
  - SBUF tiles must keep 16-byte-aligned free-dim rows; the V tile is
    padded to 1040 columns for this.
  - No warmup burst: the first real (cold) matmuls do the HAM clock ramp
    themselves (2.4GHz only after ~3.4us of continuous PE activity; cold
    = 1.2GHz). Cold pass-0 consumption self-paces with DMA delivery, and
    having no burst removes the warmup-vs-data race that re-throttled
    the clock on late-DMA runs. Long all-partition bursts also risk the
    sustained-power P0 downclock (2.0GHz whole-kernel).
  - Engine DMA queues: sync + scalar have ~150GB/s hardware-DGE rings;
    gpsimd is a ~34-85GB/s software queue. The DGE rings come up at
    ~8.9us (framework preamble); nothing big lands before ~10.5us.
  - The tensor engine pauses ~430ns every ~10.8us (firmware tick);
    unavoidable.

Input DMA schedule (arrival must match consumption). Each hwdge trigger
costs ~0.6us of engine time per 128 per-partition descriptors and the
ring admits only a few outstanding requests, so the schedule uses FEW,
GROUPED DMAs (weights are laid out partition-major per 4-f-tile group
host-side so one DMA covers an (f-group x e-range) block):
  - sync: wq f0-3 in four groups of increasing size in pass-0
    consumption order, wq f4-7 in two, then wk (2) -- 8 weight triggers
    total instead of ~50.
  - scalar: xt e-tiles 0-6 (e0 split in half for the earliest start),
    then wv (idle ring, needed only at ~69us; keeping it off sync pulls
    wk ~7us earlier for K-proj margin).
  - gpsimd: xt e-tile 7, bq/bk/ones, and the AllGather collectives.
  - Q projection runs as 2 passes of 4 f-tiles with the e-loop OUTER
    (8 PSUM banks), so pass 0 consumes xt e-tiles at ~1.7us/tile -- the
    ring delivery rate -- instead of needing all of xt in 3.4us.
  - The pair AllGather is SPLIT: each 4-f-tile half ships right after
    its Q-proj pass (stage via scalar, readback via sync hwdge rings).
    The collective runs at only ~63-86GB/s plus a partner barrier; a
    single post-Q AG had ~zero margin before the scores phase and
    stalled the PE up to 17us on jittery runs.
  - Output ou writes rotate across both hwdge rings per 256/512-col
    chunk so the 4MB drains inline with the O-phase compute.

SBUF layout (per core):
  xt [128, 8e, 1024] bf16  own 1024 rows of x[b]^T (moving + V stationary)
  w  [128, 8e, 128]  bf16  Wq^T / Wk^T stationary tiles (streamed)
  qt [128, 8f, 1024] fp8   own Q^T half; qt_g [128, 8f, 2048] gathered
  kt [128, 8f, 1024] fp8   K^T (scores stationary)
  wv [128, 8e, 1024] bf16  Wv^T (V-proj moving)
  v  [128, 8k, 1040] bf16  [V | 1 | pad] (O moving)
  pt [128, 8k, 2048] bf16  exp(scores^T) (O stationary)
"""

import numpy as np

P = 128


def _emit(nc, E=1024, S=2048, SK=1024):
    import concourse.mybir as mybir
    import concourse.tile as tile

    f32 = mybir.dt.float32
    f32r = mybir.dt.float32r
    bf16 = mybir.dt.bfloat16
    fp8 = mybir.dt.float8e4
    ACT = mybir.ActivationFunctionType

    ET = E // P     # e/f tiles (8)
    QT = S // P     # q tiles (16)
    KT = SK // P    # k tiles (8)
    NQC = S // 512  # q chunks (4)
    NKC = SK // 512  # k chunks (2)
    NFC = E // 512  # f chunks (2)

    # wq/wk are laid out partition-major per 4-f-tile group so a SINGLE DMA
    # trigger can deliver a whole (f-group x e-range) block: each hwdge
    # trigger costs ~0.6us of engine time (descriptor generation) and the
    # ring admits only a few outstanding requests, so many small triggers
    # serialize at ~0.65us each and delay the first matmul by ~3us
    xt8 = nc.dram_tensor("xt8", [ET, P, SK], bf16, kind="ExternalInput")
    wq8 = nc.dram_tensor("wq8", [2, P, 4, ET, P], bf16, kind="ExternalInput")
    wk8 = nc.dram_tensor("wk8", [2, P, 4, ET, P], bf16, kind="ExternalInput")
    wv8 = nc.dram_tensor("wv8", [P, ET, E], bf16, kind="ExternalInput")
    bq8 = nc.dram_tensor("bq8", [P, ET], f32, kind="ExternalInput")
    bk8 = nc.dram_tensor("bk8", [P, ET], f32, kind="ExternalInput")
    ones8 = nc.dram_tensor("ones8", [P, 8], bf16, kind="ExternalInput")
    ou = nc.dram_tensor("ou", [S, E], bf16, kind="ExternalOutput")
    rs = nc.dram_tensor("rs", [P, QT], f32, kind="ExternalOutput")

    groups = [[2 * i, 2 * i + 1] for i in range(4)]

    with tile.TileContext(nc) as tc:
        with (
            tc.tile_pool(name="dramp", bufs=1, space="DRAM") as dramp,
            tc.tile_pool(name="psum", bufs=8, space="PSUM") as psum,
            tc.tile_pool(name="small", bufs=1) as small,
            tc.tile_pool(name="persist", bufs=1) as pers,
            tc.tile_pool(name="obuf", bufs=3) as obp,
        ):
            # small tiles ride gpsimd: their triggers would otherwise burn
            # ~1.3us of sync trigger-head time in front of the critical wq
            ones_t = small.tile([P, 8], bf16, tag="ones")
            nc.gpsimd.dma_start(ones_t[:], ones8[:])
            bq_t = small.tile([P, ET], f32, tag="bq")
            nc.gpsimd.dma_start(bq_t[:], bq8[:])
            bk_t = small.tile([P, ET], f32, tag="bk")
            nc.gpsimd.dma_start(bk_t[:], bk8[:])
            rs_sb = small.tile([P, QT], f32, tag="rssb")

            qt_g = pers.tile([P, ET, S], fp8, tag="qtg")
            kt_t = pers.tile([P, ET, SK], fp8, tag="kt")
            v_t = pers.tile([P, KT, E + 16], bf16, tag="v")

            with (
                tc.tile_pool(name="ph1", bufs=1) as ph1,
            ):
                xt_t = ph1.tile([P, ET, SK], bf16, tag="xt")
                wv_t = ph1.tile([P, ET, E], bf16, tag="wv")
                qt_t = ph1.tile([P, ET, SK], fp8, tag="qt")
                wq_g = [
                    ph1.tile([P, 4, ET, P], bf16, tag=f"wq{g}", name=f"wq_g{g}")
                    for g in range(2)
                ]
                wk_g = [
                    ph1.tile([P, 4, ET, P], bf16, tag=f"wk{g}", name=f"wk_g{g}")
                    for g in range(2)
                ]
                # per-half AG staging: the pair AllGather runs at only
                # ~63-86GB/s plus a partner-sync barrier, so one 1MB AG
                # issued after the full Q projection has ~zero margin before
                # the scores phase needs qt_g (17us PE stall on unlucky
                # runs). Shipping each 4-f-tile half as soon as its Q-proj
                # pass finishes starts the exchange ~16us earlier.
                qh_h = [
                    dramp.tile(
                        [P, ET // 2, SK], fp8, tag=f"qh{h}", name=f"qh_d{h}"
                    )
                    for h in range(2)
                ]
                qg_h = [
                    dramp.tile(
                        [2, P, ET // 2, SK], fp8, tag=f"qg{h}", name=f"qg_d{h}"
                    )
                    for h in range(2)
                ]

                # (no warmup burst: the first real matmuls do the HAM clock
                # ramp themselves -- cold pass-0 consumption, 8 MMs/e-tile at
                # 430ns, self-paces with the ~1.7us/e-tile DMA delivery, and
                # dropping the burst removes the warmup-vs-data race that
                # re-throttled the clock on late-DMA runs)

                # ---- input DMA ----
                # sync: wq f0-3 in four grouped (all-f x e-pair) DMAs in
                #   pass-0 consumption order, then wq f4-7 / wk / wv as one
                #   big DMA each -- 8 triggers total instead of ~50.
                # scalar: xt e-tiles 0-6 e-major (matches the Q-proj pass-0
                #   e-outer consumption rate of ~1.7us/e-tile); gpsimd takes
                #   xt e-tile 7 + the small tiles.
                # first two e-pair groups are small for the earliest start;
                # later blocks use e-quad groups (1KB descriptors, ~2x the
                # generation efficiency) so the serial ring stays ahead of
                # the pass-0/pass-1 consumption rate
                nc.sync.dma_start(wq_g[0][:, 0:2, 0:2, :], wq8[0][:, 0:2, 0:2, :])
                nc.sync.dma_start(wq_g[0][:, 2:4, 0:2, :], wq8[0][:, 2:4, 0:2, :])
                nc.sync.dma_start(wq_g[0][:, :, 2:4, :], wq8[0][:, :, 2:4, :])
                nc.sync.dma_start(wq_g[0][:, :, 4:8, :], wq8[0][:, :, 4:8, :])
                nc.sync.dma_start(wq_g[1][:, :, 0:4, :], wq8[1][:, :, 0:4, :])
                nc.sync.dma_start(wq_g[1][:, :, 4:8, :], wq8[1][:, :, 4:8, :])
                nc.sync.dma_start(wk_g[0][:], wk8[0])
                nc.sync.dma_start(wk_g[1][:], wk8[1])
                # xt: full e-tile transfers (2KB contiguous per partition); e0
                # is split so its first 512 columns land with the first wq
                nc.scalar.dma_start(xt_t[:, 0, 0:512], xt8[0, :, 0:512])
                nc.scalar.dma_start(xt_t[:, 0, 512:SK], xt8[0, :, 512:SK])
                for e in range(1, ET):
                    ring = nc.gpsimd if e == ET - 1 else nc.scalar
                    ring.dma_start(xt_t[:, e], xt8[e])
                # wv rides scalar after xt (idle ring, needed only at ~69us);
                # keeping it off sync pulls wk ~7us earlier for K-proj margin
                nc.scalar.dma_start(wv_t[:], wv8[:])

                # ---- Q projection, own query half only (the first SK
                # permuted columns = global queries [h*SK,(h+1)*SK)).
                # 2 passes of 4 f-tiles with the e-loop OUTER: pass 0 consumes
                # xt e-tiles at ~1.7us/tile, the rate the DMA ring delivers
                # them, so the PE starts ~6us earlier without starving ----
                for half in range(2):
                    fb = 4 * half
                    psq = [
                        psum.tile([P, 512], f32, tag="mm", name=f"q{fb}_{fi}{qc}")
                        for fi in range(4)
                        for qc in range(NKC)
                    ]
                    for e in range(ET):
                        for fi in range(4):
                            for qc in range(NKC):
                                nc.tensor.matmul(
                                    psq[fi * NKC + qc][:],
                                    wq_g[half][:, fi, e],
                                    xt_t[:, e, qc * 512 : (qc + 1) * 512],
                                    start=(e == 0),
                                    stop=(e == ET - 1),
                                )
                    for fi in range(4):
                        for qc in range(NKC):
                            nc.scalar.add(
                                qt_t[:, fb + fi, qc * 512 : (qc + 1) * 512],
                                psq[fi * NKC + qc][:],
                                bq_t[:, fb + fi : fb + fi + 1],
                            )
                    # stage this half to DRAM, pair-allgather into global
                    # query order, read back both cores' copies; overlaps
                    # with the rest of Q-proj and the K/V projections. Stage
                    # + readback ride the fast hwdge rings (scalar / sync,
                    # both idle here) -- gpsimd's ~34GB/s software queue was
                    # itself a several-us race
                    nc.scalar.dma_start(
                        qh_h[half][:], qt_t[:, fb : fb + 4]
                    )
                    nc.gpsimd.collective_compute(
                        "AllGather",
                        mybir.AluOpType.bypass,
                        replica_groups=groups,
                        ins=[qh_h[half][:]],
                        outs=[qg_h[half][:]],
                    )
                    for g in range(2):
                        nc.sync.dma_start(
                            qt_g[:, fb : fb + 4, g * SK : (g + 1) * SK],
                            qg_h[half][g],
                        )

                # ---- K projection (key half = first SK columns of xt) ----
                for f in range(ET):
                    ps2 = [
                        psum.tile([P, 512], f32, tag="mm", name=f"k{f}_{kc}")
                        for kc in range(NKC)
                    ]
                    for e in range(ET):
                        for kc in range(NKC):
                            nc.tensor.matmul(
                                ps2[kc][:],
                                wk_g[f // 4][:, f % 4, e],
                                xt_t[:, e, kc * 512 : (kc + 1) * 512],
                                start=(e == 0),
                                stop=(e == ET - 1),
                            )
                    for kc in range(NKC):
                        nc.scalar.add(
                            kt_t[:, f, kc * 512 : (kc + 1) * 512],
                            ps2[kc][:],
                            bk_t[:, f : f + 1],
                        )

                # ---- V projection: v[k, f] = sum_e xt[e, k] * wv[e, f] ----
                for kt in range(KT):
                    ps2 = [
                        psum.tile([P, 512], f32, tag="mm", name=f"v{kt}_{fc}")
                        for fc in range(NFC)
                    ]
                    for e in range(ET):
                        for fc in range(NFC):
                            nc.tensor.matmul(
                                ps2[fc][:],
                                xt_t[:, e, kt * P : (kt + 1) * P],
                                wv_t[:, e, fc * 512 : (fc + 1) * 512],
                                start=(e == 0),
                                stop=(e == ET - 1),
                            )
                    for fc in range(NFC):
                        nc.vector.tensor_copy(
                            v_t[:, kt, fc * 512 : (fc + 1) * 512], ps2[fc][:]
                        )
                    nc.vector.tensor_copy(v_t[:, kt, E : E + 1], ones_t[:, 0:1])

            with tc.tile_pool(name="ptp", bufs=1) as ptp:
                pt_t = ptp.tile([P, KT, S], bf16, tag="pt")

                # ---- scores^T (fp8 DoubleRow) + exp; rowsums in a second
                # pass so the PE never waits on the scalar exp ----
                # 4 q-chunks per stationary (vs 2): halves the DR LDWEIGHTS
                # exposure per 512-col matmul
                DR = mybir.MatmulPerfMode.DoubleRow
                NQC4 = S // 512
                scale = float(1.0 / np.sqrt(np.float32(E)))
                for kt in range(KT):
                    ps4 = [
                        psum.tile([P, 512], f32, tag="mm", name=f"s{kt}_{qc}")
                        for qc in range(NQC4)
                    ]
                    for fp in range(ET // 2):
                        for qc in range(NQC4):
                            nc.tensor.matmul(
                                ps4[qc][:],
                                kt_t[:, 2 * fp : 2 * fp + 2, kt * P : (kt + 1) * P],
                                qt_g[:, 2 * fp : 2 * fp + 2, qc * 512 : (qc + 1) * 512],
                                start=(fp == 0),
                                stop=(fp == ET // 2 - 1),
                                perf_mode=DR,
                            )
                    for qc in range(NQC4):
                        nc.scalar.activation(
                            pt_t[:, kt, qc * 512 : (qc + 1) * 512], ps4[qc][:],
                            ACT.Exp, scale=scale,
                        )

                # ---- O = pt^T @ v, unnormalized; store bf16 ----
                # output DMA rotates across both hwdge rings per chunk so the
                # 4MB of ou writes drain inline with compute; each chunk's
                # DMA fires as soon as its copy lands (short final drain)
                CH = [(0, 512), (512, 768), (768, E + 1)]
                out_rings = [nc.sync, nc.scalar]
                for qt in range(QT):
                    po = [
                        psum.tile([P, 512], f32, tag="mm", name=f"o{qt}_{fc}")
                        for fc in range(len(CH))
                    ]
                    for kt in range(KT):
                        for fc, (c0, c1) in enumerate(CH):
                            nc.tensor.matmul(
                                po[fc][:, : c1 - c0],
                                pt_t[:, kt, qt * P : (qt + 1) * P],
                                v_t[:, kt, c0:c1],
                                start=(kt == 0),
                                stop=(kt == KT - 1),
                            )
                    o_sb = obp.tile([P, E], bf16, tag="ob")
                    nc.vector.tensor_copy(o_sb[:, 0:512], po[0][:])
                    out_rings[qt % 2].dma_start(
                        ou[qt * P : (qt + 1) * P, 0:512], o_sb[:, 0:512]
                    )
                    nc.vector.tensor_copy(o_sb[:, 512:768], po[1][:, :256])
                    out_rings[(qt + 1) % 2].dma_start(
                        ou[qt * P : (qt + 1) * P, 512:768], o_sb[:, 512:768]
                    )
                    nc.vector.tensor_copy(o_sb[:, 768:E], po[2][:, :256])
                    nc.vector.tensor_copy(
                        rs_sb[:, qt : qt + 1], po[2][:, 256:257]
                    )
                    out_rings[qt % 2].dma_start(
                        ou[qt * P : (qt + 1) * P, 768:E], o_sb[:, 768:E]
                    )
                nc.sync.dma_start(rs[:], rs_sb[:])


_NC_CACHE = {}


def build_nc(E=1024, S=2048, SK=1024):
    key = (E, S, SK)
    if key in _NC_CACHE:
        return _NC_CACHE[key]
    import concourse.bacc as bacc

    nc = bacc.Bacc(None, target_bir_lowering=False)
    _emit(nc, E=E, S=S, SK=SK)
    nc.finalize()
    _NC_CACHE[key] = nc
    return nc


def _round_f32r(a):
    """Round fp32 to fp32r (tf32-like: 11 explicit mantissa bits, RNE)."""
    u = np.ascontiguousarray(a, np.float32).view(np.uint32)
    u = u + np.uint32(0x7FF) + ((u >> np.uint32(12)) & np.uint32(1))
    return (u & np.uint32(0xFFFFF000)).view(np.float32)


def make_in_maps(x, Wq, bq, Wk, bk, Wv, bv, E=1024, S=2048, SK=1024):
    """Host-side prep: per-core input dicts for run_bass_kernel_spmd."""
    import ml_dtypes

    bf16 = ml_dtypes.bfloat16
    ET = E // P
    scale = np.float32(1.0 / np.sqrt(np.float32(E)))
    x = np.asarray(x, np.float32)
    B = x.shape[0]
    n_half = S // SK

    def wtile(w):
        # [group, p(e), f_in_group, e_tile, c(f)] stationary blocks,
        # partition-major per 4-f-tile group so one DMA covers a whole
        # (f-group x e-range) block with matching iteration order
        return np.ascontiguousarray(
            np.asarray(w, np.float32)
            .reshape(2, 4, P, ET, P)
            .transpose(0, 4, 1, 3, 2)
        ).astype(bf16)

    wq8 = wtile(Wq)
    wk8 = wtile(Wk)
    # wv8[p, e, f] = Wv[f, e*128+p] (partition-major for a single DMA)
    wv8 = np.ascontiguousarray(
        np.asarray(Wv, np.float32).T.reshape(ET, P, E).transpose(1, 0, 2)
    ).astype(bf16)
    bq8 = np.ascontiguousarray(np.asarray(bq, np.float32).reshape(ET, P).T)
    bk8 = np.ascontiguousarray(np.asarray(bk, np.float32).reshape(ET, P).T)
    ones8 = np.ones((P, 8), bf16)

    in_maps = []
    for c in range(B * n_half):
        b, h = divmod(c, n_half)
        xt_half = x[b].T[:, h * SK : (h + 1) * SK]  # [E, SK]
        xt8 = np.ascontiguousarray(xt_half.reshape(ET, P, SK)).astype(bf16)
        in_maps.append(
            {
                "xt8": xt8,
                "wq8": wq8,
                "wk8": wk8,
                "wv8": wv8,
                "bq8": bq8,
                "bk8": bk8,
                "ones8": ones8,
            }
        )
    return in_maps


def kernel(x, Wq, bq, Wk, bk, Wv, bv):
    from concourse.bass_utils import run_bass_kernel_spmd

    E, S, SK = 1024, 2048, 1024
    x = np.asarray(x, np.float32)
    B = x.shape[0]
    n_half = S // SK
    nc = build_nc(E=E, S=S, SK=SK)
    in_maps = make_in_maps(x, Wq, bq, Wk, bk, Wv, bv, E=E, S=S, SK=SK)
    n_cores = len(in_maps)
    res = run_bass_kernel_spmd(nc, in_maps, list(range(n_cores)))

    bvf = np.asarray(bv, np.float32)
    out = np.empty((B, S, E), np.float32)
    for b in range(B):
        osum = None
        rsum = None
        for h in range(n_half):
            r = res.results[b * n_half + h]
            o_h = np.asarray(r["ou"]).astype(np.float32)
            rs_h = np.asarray(r["rs"]).astype(np.float32).T.reshape(S)
            osum = o_h if osum is None else osum + o_h
            rsum = rs_h if rsum is None else rsum + rs_h
        out[b] = osum / rsum[:, None] + bvf[None, :]
    return out



# revision 61
# speedup vs baseline: 1.0435x; 1.0359x over previous
"""Single-head attention (B=4, S=2048, E=1024, fp32) on 8 trn2 NeuronCores.

Sharding: (batch, key-half) -> 8 shards. Core c handles batch c//2 and half
h = c%2: keys/values AND queries [h*1024, (h+1)*1024) of x[b]. Per core:
  - Q/K/V projections for its own 1024 rows only (Wq/Wk applied without the
    1/sqrt(E) scale; outputs quantized straight to fp8e4m3, which needs the
    natural value range -- the scale is applied later inside the exp).
  - Q^T halves are exchanged within each core pair via a DRAM AllGather
    (replica_groups [[0,1],[2,3],[4,5],[6,7]]), landing in global query
    order; the exchange overlaps the K and V projections.
  - scores^T = K^T-stationary x Q^T-moving in fp8 DoubleRow perf mode
    (2 f-tiles contracted per matmul at 0.5 cycles/row), exp via the scalar
    engine (scale=1/32) into a bf16 pt tile kept resident in SBUF.
  - V carries an extra ones column (padded tile), so the O matmul
    O_h = pt^T @ [V_h | 1] yields the softmax denominators rs_h as column
    1024 -- no separate rowsum pass on the PE.
  - Outputs: unnormalized O_h (bf16) + rs_h; the host combines
    out = (O_0 + O_1) / (rs_0 + rs_1) + bv  (the V bias commutes with the
    softmax average, so it is added once on the host).

Matmul dtype/perf notes (measured on HW):
  - bf16 matmuls hit the 215ns/512-col streaming floor ONLY when the
    stationary operand is reused across consecutive matmuls; swapping the
    stationary every matmul exposes ~45ns of FWL LDWEIGHTS. All loops keep
    each stationary for 2-4 consecutive matmuls.
  - SBUF tiles must keep 16-byte-aligned free-dim rows; the V tile is
    padded to 1040 columns for this.
  - No warmup burst: the first real (cold) matmuls do the HAM clock ramp
    themselves (2.4GHz only after ~3.4us of continuous PE activity; cold
    = 1.2GHz). Cold pass-0 consumption self-paces with DMA delivery, and
    having no burst removes the warmup-vs-data race that re-throttled
    the clock on late-DMA runs. Long all-partition bursts also risk the
    sustained-power P0 downclock (2.0GHz whole-kernel).
  - Engine DMA queues: sync + scalar have ~150GB/s hardware-DGE rings;
    gpsimd is a ~34-85GB/s software queue. The DGE rings come up at
    ~8.9us (framework preamble); nothing big lands before ~10.5us.
  - The tensor engine pauses ~430ns every ~10.8us (firmware tick);
    unavoidable.

Input DMA schedule (arrival must match consumption). Each hwdge trigger
costs ~0.6us of engine time per 128 per-partition descriptors and the
ring admits only a few outstanding requests, so the schedule uses FEW,
GROUPED DMAs (weights are laid out partition-major per 4-f-tile group
host-side so one DMA covers an (f-group x e-range) block):
  - sync: wq f0-3 in four groups of increasing size in pass-0
    consumption order, wq f4-7 in two, then wk (2) -- 8 weight triggers
    total instead of ~50.
  - scalar: xt e-tiles 0-6 (e0 split in half for the earliest start),
    then wv (idle ring, needed only at ~69us; keeping it off sync pulls
    wk ~7us earlier for K-proj margin).
  - gpsimd: xt e-tile 7, bq/bk/ones, and the AllGather collectives.
  - Q projection runs as 2 passes of 4 f-tiles with the e-loop OUTER
    (8 PSUM banks), so pass 0 consumes xt e-tiles at ~1.7us/tile -- the
    ring delivery rate -- instead of needing all of xt in 3.4us.
  - The pair AllGather is SPLIT: each 4-f-tile half ships right after
    its Q-proj pass (stage via scalar, readback via sync hwdge rings).
    The collective runs at only ~63-86GB/s plus a partner barrier; a
    single post-Q AG had ~zero margin before the scores phase and
    stalled the PE up to 17us on jittery runs.
  - Output ou writes rotate across both hwdge rings per 256/512-col
    chunk so the 4MB drains inline with the O-phase compute.

SBUF layout (per core):
  xt [128, 8e, 1024] bf16  own 1024 rows of x[b]^T (moving + V stationary)
  w  [128, 8e, 128]  bf16  Wq^T / Wk^T stationary tiles (streamed)
  qt [128, 8f, 1024] fp8   own Q^T half; qt_g [128, 8f, 2048] gathered
  kt [128, 8f, 1024] fp8   K^T (scores stationary)
  wv [128, 8e, 1024] bf16  Wv^T (V-proj moving)
  v  [128, 8k, 1040] bf16  [V | 1 | pad] (O moving)
  pt [128, 8k, 2048] bf16  exp(scores^T) (O stationary)
"""

import numpy as np

P = 128


def _emit(nc, E=1024, S=2048, SK=1024):
    import concourse.mybir as mybir
    import concourse.tile as tile

    f32 = mybir.dt.float32
    f32r = mybir.dt.float32r
    bf16 = mybir.dt.bfloat16
    fp8 = mybir.dt.float8e4
    ACT = mybir.ActivationFunctionType

    ET = E // P     # e/f tiles (8)
    QT = S // P     # q tiles (16)
    KT = SK // P    # k tiles (8)
    NQC = S // 512  # q chunks (4)
    NKC = SK // 512  # k chunks (2)
    NFC = E // 512  # f chunks (2)

    # wq/wk are laid out partition-major per 4-f-tile group so a SINGLE DMA
    # trigger can deliver a whole (f-group x e-range) block: each hwdge
    # trigger costs ~0.6us of engine time (descriptor generation) and the
    # ring admits only a few outstanding requests, so many small triggers
    # serialize at ~0.65us each and delay the first matmul by ~3us
    xt8 = nc.dram_tensor("xt8", [ET, P, SK], bf16, kind="ExternalInput")
    wq8 = nc.dram_tensor("wq8", [2, P, 4, ET, P], bf16, kind="ExternalInput")
    wk8 = nc.dram_tensor("wk8", [2, P, 4, ET, P], bf16, kind="ExternalInput")
    wv8 = nc.dram_tensor("wv8", [P, ET, E], bf16, kind="ExternalInput")
    bq8 = nc.dram_tensor("bq8", [P, ET], f32, kind="ExternalInput")
    bk8 = nc.dram_tensor("bk8", [P, ET], f32, kind="ExternalInput")
    ones8 = nc.dram_tensor("ones8", [P, 8], bf16, kind="ExternalInput")
    ou = nc.dram_tensor("ou", [S, E], bf16, kind="ExternalOutput")
    rs = nc.dram_tensor("rs", [P, QT], f32, kind="ExternalOutput")

    groups = [[2 * i, 2 * i + 1] for i in range(4)]

    with tile.TileContext(nc) as tc:
        with (
            tc.tile_pool(name="dramp", bufs=1, space="DRAM") as dramp,
            tc.tile_pool(name="psum", bufs=8, space="PSUM") as psum,
            tc.tile_pool(name="small", bufs=1) as small,
            tc.tile_pool(name="persist", bufs=1) as pers,
            tc.tile_pool(name="obuf", bufs=3) as obp,
        ):
            # small tiles ride gpsimd: their triggers would otherwise burn
            # ~1.3us of sync trigger-head time in front of the critical wq
            ones_t = small.tile([P, 8], bf16, tag="ones")
            nc.gpsimd.dma_start(ones_t[:], ones8[:])
            bq_t = small.tile([P, ET], f32, tag="bq")
            nc.gpsimd.dma_start(bq_t[:], bq8[:])
            bk_t = small.tile([P, ET], f32, tag="bk")
            nc.gpsimd.dma_start(bk_t[:], bk8[:])
            rs_sb = small.tile([P, QT], f32, tag="rssb")

            qt_g = pers.tile([P, ET, S], fp8, tag="qtg")
            kt_t = pers.tile([P, ET, SK], fp8, tag="kt")
            v_t = pers.tile([P, KT, E + 16], bf16, tag="v")

            with (
                tc.tile_pool(name="ph1", bufs=1) as ph1,
            ):
                xt_t = ph1.tile([P, ET, SK], bf16, tag="xt")
                wv_t = ph1.tile([P, ET, E], bf16, tag="wv")
                qt_t = ph1.tile([P, ET, SK], fp8, tag="qt")
                wq_g = [
                    ph1.tile([P, 4, ET, P], bf16, tag=f"wq{g}", name=f"wq_g{g}")
                    for g in range(2)
                ]
                wk_g = [
                    ph1.tile([P, 4, ET, P], bf16, tag=f"wk{g}", name=f"wk_g{g}")
                    for g in range(2)
                ]
                # per-half AG staging: the pair AllGather runs at only
                # ~63-86GB/s plus a partner-sync barrier, so one 1MB AG
                # issued after the full Q projection has ~zero margin before
                # the scores phase needs qt_g (17us PE stall on unlucky
                # runs). Shipping each 4-f-tile half as soon as its Q-proj
                # pass finishes starts the exchange ~16us earlier.
                qh_h = [
                    dramp.tile(
                        [P, ET // 2, SK], fp8, tag=f"qh{h}", name=f"qh_d{h}"
                    )
                    for h in range(2)
                ]
                qg_h = [
                    dramp.tile(
                        [2, P, ET // 2, SK], fp8, tag=f"qg{h}", name=f"qg_d{h}"
                    )
                    for h in range(2)
                ]

                # (no warmup burst: the first real matmuls do the HAM clock
                # ramp themselves -- cold pass-0 consumption, 8 MMs/e-tile at
                # 430ns, self-paces with the ~1.7us/e-tile DMA delivery, and
                # dropping the burst removes the warmup-vs-data race that
                # re-throttled the clock on late-DMA runs)

                # ---- input DMA ----
                # sync: wq f0-3 in four grouped (all-f x e-pair) DMAs in
                #   pass-0 consumption order, then wq f4-7 / wk / wv as one
                #   big DMA each -- 8 triggers total instead of ~50.
                # scalar: xt e-tiles 0-6 e-major (matches the Q-proj pass-0
                #   e-outer consumption rate of ~1.7us/e-tile); gpsimd takes
                #   xt e-tile 7 + the small tiles.
                # first two e-pair groups are small for the earliest start;
                # later blocks use e-quad groups (1KB descriptors, ~2x the
                # generation efficiency) so the serial ring stays ahead of
                # the pass-0/pass-1 consumption rate
                nc.sync.dma_start(wq_g[0][:, 0:2, 0:2, :], wq8[0][:, 0:2, 0:2, :])
                nc.sync.dma_start(wq_g[0][:, 2:4, 0:2, :], wq8[0][:, 2:4, 0:2, :])
                nc.sync.dma_start(wq_g[0][:, :, 2:4, :], wq8[0][:, :, 2:4, :])
                nc.sync.dma_start(wq_g[0][:, :, 4:8, :], wq8[0][:, :, 4:8, :])
                nc.sync.dma_start(wq_g[1][:, :, 0:4, :], wq8[1][:, :, 0:4, :])
                nc.sync.dma_start(wk_g[0][:], wk8[0])
                nc.sync.dma_start(wk_g[1][:], wk8[1])
                # xt: full e-tile transfers (2KB contiguous per partition); e0
                # is split so its first 512 columns land with the first wq
                nc.scalar.dma_start(xt_t[:, 0, 0:512], xt8[0, :, 0:512])
                nc.scalar.dma_start(xt_t[:, 0, 512:SK], xt8[0, :, 512:SK])
                for e in range(1, ET):
                    ring = nc.gpsimd if e == ET - 1 else nc.scalar
                    ring.dma_start(xt_t[:, e], xt8[e])
                # the last wq group (f4-7 x e4-7, latest need-time ~31us) and
                # wv ride scalar after xt: sync's early queue shrinks by
                # 512KB so every wq/wk piece lands ~3.4us earlier -- protects
                # the schedule when the DMA rings run slow (hot device)
                nc.scalar.dma_start(
                    wq_g[1][:, :, 4:8, :], wq8[1][:, :, 4:8, :]
                )
                nc.scalar.dma_start(wv_t[:], wv8[:])

                # ---- Q projection, own query half only (the first SK
                # permuted columns = global queries [h*SK,(h+1)*SK)).
                # 2 passes of 4 f-tiles with the e-loop OUTER: pass 0 consumes
                # xt e-tiles at ~1.7us/tile, the rate the DMA ring delivers
                # them, so the PE starts ~6us earlier without starving ----
                for half in range(2):
                    fb = 4 * half
                    psq = [
                        psum.tile([P, 512], f32, tag="mm", name=f"q{fb}_{fi}{qc}")
                        for fi in range(4)
                        for qc in range(NKC)
                    ]
                    for e in range(ET):
                        for fi in range(4):
                            for qc in range(NKC):
                                nc.tensor.matmul(
                                    psq[fi * NKC + qc][:],
                                    wq_g[half][:, fi, e],
                                    xt_t[:, e, qc * 512 : (qc + 1) * 512],
                                    start=(e == 0),
                                    stop=(e == ET - 1),
                                )
                    for fi in range(4):
                        for qc in range(NKC):
                            nc.scalar.add(
                                qt_t[:, fb + fi, qc * 512 : (qc + 1) * 512],
                                psq[fi * NKC + qc][:],
                                bq_t[:, fb + fi : fb + fi + 1],
                            )
                    # stage this half to DRAM, pair-allgather into global
                    # query order, read back both cores' copies; overlaps
                    # with the rest of Q-proj and the K/V projections. Stage
                    # + readback ride the fast hwdge rings (scalar / sync,
                    # both idle here) -- gpsimd's ~34GB/s software queue was
                    # itself a several-us race
                    nc.scalar.dma_start(
                        qh_h[half][:], qt_t[:, fb : fb + 4]
                    )
                    nc.gpsimd.collective_compute(
                        "AllGather",
                        mybir.AluOpType.bypass,
                        replica_groups=groups,
                        ins=[qh_h[half][:]],
                        outs=[qg_h[half][:]],
                    )
                    for g in range(2):
                        nc.sync.dma_start(
                            qt_g[:, fb : fb + 4, g * SK : (g + 1) * SK],
                            qg_h[half][g],
                        )

                # ---- K projection (key half = first SK columns of xt) ----
                for f in range(ET):
                    ps2 = [
                        psum.tile([P, 512], f32, tag="mm", name=f"k{f}_{kc}")
                        for kc in range(NKC)
                    ]
                    for e in range(ET):
                        for kc in range(NKC):
                            nc.tensor.matmul(
                                ps2[kc][:],
                                wk_g[f // 4][:, f % 4, e],
                                xt_t[:, e, kc * 512 : (kc + 1) * 512],
                                start=(e == 0),
                                stop=(e == ET - 1),
                            )
                    for kc in range(NKC):
                        nc.scalar.add(
                            kt_t[:, f, kc * 512 : (kc + 1) * 512],
                            ps2[kc][:],
                            bk_t[:, f : f + 1],
                        )

                # ---- V projection: v[k, f] = sum_e xt[e, k] * wv[e, f] ----
                for kt in range(KT):
                    ps2 = [
                        psum.tile([P, 512], f32, tag="mm", name=f"v{kt}_{fc}")
                        for fc in range(NFC)
                    ]
                    for e in range(ET):
                        for fc in range(NFC):
                            nc.tensor.matmul(
                                ps2[fc][:],
                                xt_t[:, e, kt * P : (kt + 1) * P],
                                wv_t[:, e, fc * 512 : (fc + 1) * 512],
                                start=(e == 0),
                                stop=(e == ET - 1),
                            )
                    for fc in range(NFC):
                        nc.vector.tensor_copy(
                            v_t[:, kt, fc * 512 : (fc + 1) * 512], ps2[fc][:]
                        )
                    nc.vector.tensor_copy(v_t[:, kt, E : E + 1], ones_t[:, 0:1])

            with tc.tile_pool(name="ptp", bufs=1) as ptp:
                pt_t = ptp.tile([P, KT, S], bf16, tag="pt")

                # ---- scores^T (fp8 DoubleRow) + exp; rowsums in a second
                # pass so the PE never waits on the scalar exp ----
                # 4 q-chunks per stationary (vs 2): halves the DR LDWEIGHTS
                # exposure per 512-col matmul
                DR = mybir.MatmulPerfMode.DoubleRow
                NQC4 = S // 512
                scale = float(1.0 / np.sqrt(np.float32(E)))
                for kt in range(KT):
                    ps4 = [
                        psum.tile([P, 512], f32, tag="mm", name=f"s{kt}_{qc}")
                        for qc in range(NQC4)
                    ]
                    for fp in range(ET // 2):
                        for qc in range(NQC4):
                            nc.tensor.matmul(
                                ps4[qc][:],
                                kt_t[:, 2 * fp : 2 * fp + 2, kt * P : (kt + 1) * P],
                                qt_g[:, 2 * fp : 2 * fp + 2, qc * 512 : (qc + 1) * 512],
                                start=(fp == 0),
                                stop=(fp == ET // 2 - 1),
                                perf_mode=DR,
                            )
                    for qc in range(NQC4):
                        nc.scalar.activation(
                            pt_t[:, kt, qc * 512 : (qc + 1) * 512], ps4[qc][:],
                            ACT.Exp, scale=scale,
                        )

                # ---- O = pt^T @ v, unnormalized; store bf16 ----
                # output DMA rotates across both hwdge rings per chunk so the
                # 4MB of ou writes drain inline with compute; each chunk's
                # DMA fires as soon as its copy lands (short final drain)
                CH = [(0, 512), (512, 768), (768, E + 1)]
                out_rings = [nc.sync, nc.scalar]
                for qt in range(QT):
                    po = [
                        psum.tile([P, 512], f32, tag="mm", name=f"o{qt}_{fc}")
                        for fc in range(len(CH))
                    ]
                    for kt in range(KT):
                        for fc, (c0, c1) in enumerate(CH):
                            nc.tensor.matmul(
                                po[fc][:, : c1 - c0],
                                pt_t[:, kt, qt * P : (qt + 1) * P],
                                v_t[:, kt, c0:c1],
                                start=(kt == 0),
                                stop=(kt == KT - 1),
                            )
                    o_sb = obp.tile([P, E], bf16, tag="ob")
                    nc.vector.tensor_copy(o_sb[:, 0:512], po[0][:])
                    out_rings[qt % 2].dma_start(
                        ou[qt * P : (qt + 1) * P, 0:512], o_sb[:, 0:512]
                    )
                    nc.vector.tensor_copy(o_sb[:, 512:768], po[1][:, :256])
                    out_rings[(qt + 1) % 2].dma_start(
                        ou[qt * P : (qt + 1) * P, 512:768], o_sb[:, 512:768]
                    )
                    nc.vector.tensor_copy(o_sb[:, 768:E], po[2][:, :256])
                    nc.vector.tensor_copy(
                        rs_sb[:, qt : qt + 1], po[2][:, 256:257]
                    )
                    out_rings[qt % 2].dma_start(
                        ou[qt * P : (qt + 1) * P, 768:E], o_sb[:, 768:E]
                    )
                nc.sync.dma_start(rs[:], rs_sb[:])


_NC_CACHE = {}


def build_nc(E=1024, S=2048, SK=1024):
    key = (E, S, SK)
    if key in _NC_CACHE:
        return _NC_CACHE[key]
    import concourse.bacc as bacc

    nc = bacc.Bacc(None, target_bir_lowering=False)
    _emit(nc, E=E, S=S, SK=SK)
    nc.finalize()
    _NC_CACHE[key] = nc
    return nc


def _round_f32r(a):
    """Round fp32 to fp32r (tf32-like: 11 explicit mantissa bits, RNE)."""
    u = np.ascontiguousarray(a, np.float32).view(np.uint32)
    u = u + np.uint32(0x7FF) + ((u >> np.uint32(12)) & np.uint32(1))
    return (u & np.uint32(0xFFFFF000)).view(np.float32)


def make_in_maps(x, Wq, bq, Wk, bk, Wv, bv, E=1024, S=2048, SK=1024):
    """Host-side prep: per-core input dicts for run_bass_kernel_spmd."""
    import ml_dtypes

    bf16 = ml_dtypes.bfloat16
    ET = E // P
    scale = np.float32(1.0 / np.sqrt(np.float32(E)))
    x = np.asarray(x, np.float32)
    B = x.shape[0]
    n_half = S // SK

    def wtile(w):
        # [group, p(e), f_in_group, e_tile, c(f)] stationary blocks,
        # partition-major per 4-f-tile group so one DMA covers a whole
        # (f-group x e-range) block with matching iteration order
        return np.ascontiguousarray(
            np.asarray(w, np.float32)
            .reshape(2, 4, P, ET, P)
            .transpose(0, 4, 1, 3, 2)
        ).astype(bf16)

    wq8 = wtile(Wq)
    wk8 = wtile(Wk)
    # wv8[p, e, f] = Wv[f, e*128+p] (partition-major for a single DMA)
    wv8 = np.ascontiguousarray(
        np.asarray(Wv, np.float32).T.reshape(ET, P, E).transpose(1, 0, 2)
    ).astype(bf16)
    bq8 = np.ascontiguousarray(np.asarray(bq, np.float32).reshape(ET, P).T)
    bk8 = np.ascontiguousarray(np.asarray(bk, np.float32).reshape(ET, P).T)
    ones8 = np.ones((P, 8), bf16)

    in_maps = []
    for c in range(B * n_half):
        b, h = divmod(c, n_half)
        xt_half = x[b].T[:, h * SK : (h + 1) * SK]  # [E, SK]
        xt8 = np.ascontiguousarray(xt_half.reshape(ET, P, SK)).astype(bf16)
        in_maps.append(
            {
                "xt8": xt8,
                "wq8": wq8,
                "wk8": wk8,
                "wv8": wv8,
                "bq8": bq8,
                "bk8": bk8,
                "ones8": ones8,
            }
        )
    return in_maps


def kernel(x, Wq, bq, Wk, bk, Wv, bv):
    from concourse.bass_utils import run_bass_kernel_spmd

    E, S, SK = 1024, 2048, 1024
    x = np.asarray(x, np.float32)
    B = x.shape[0]
    n_half = S // SK
    nc = build_nc(E=E, S=S, SK=SK)
    in_maps = make_in_maps(x, Wq, bq, Wk, bk, Wv, bv, E=E, S=S, SK=SK)
    n_cores = len(in_maps)
    res = run_bass_kernel_spmd(nc, in_maps, list(range(n_cores)))

    bvf = np.asarray(bv, np.float32)
    out = np.empty((B, S, E), np.float32)
    for b in range(B):
        osum = None
        rsum = None
        for h in range(n_half):
            r = res.results[b * n_half + h]
            o_h = np.asarray(r["ou"]).astype(np.float32)
            rs_h = np.asarray(r["rs"]).astype(np.float32).T.reshape(S)
            osum = o_h if osum is None else osum + o_h
            rsum = rs_h if rsum is None else rsum + rs_h
        out[b] = osum / rsum[:, None] + bvf[None, :]
    return out

